# revision 6
# baseline (speedup 1.0000x reference)
"""Trainium2 Bass kernel for nn_KeypointStreamEncoder (masked tanh-MHA + global mix).

Strategy: data-parallel over batch (B=8 -> one sample per NeuronCore, no
collectives). Per sample, the ~50% key/query mask is exploited by compacting
tokens on the host (gather of valid rows + zero-pad to a multiple of 128), so
the O(L^2) attention work runs only on valid tokens.

Math restructure (exact up to fp32 rounding):
  - scores = tanh(q k^T / 8) in [-1, 1], so the global amax subtraction in
    softmax cancels exactly (exp(s - m) / sum exp(s - m) == exp(s) / sum exp(s))
    and is skipped; exp stays in [e^-1, e].
  - weights renormalization pass (sum == 1) is an identity and is skipped.
  - attention output = (0.5/T) * (E @ v) + b_q * (expG^T-matmul) where
    T = sum of masked exp-scores over (heads, q, k); the per-q global-branch
    scale b_q = 0.5 * colscale_q / gsum commutes through the output projection
    as a per-row scale.
  - softmax(G) row denominators are computed without max subtraction
    (|G| ~ N(0,1), exp cannot overflow); column gathers of G are host-side.

All device matmuls are fp32. Head dim D=64 matmuls are packed two-per-PE-pass
via tile_position row/column tiling.
"""

import math

import numpy as np

import concourse.mybir as mybir
import concourse.tile as tile
from concourse import bacc
from concourse.bass_utils import run_bass_kernel_spmd
from concourse.masks import make_identity

H = 8
D = 64
E = 512
SCALE = 1.0 / 8.0  # 1/sqrt(D)
MIX = 0.5
L_FULL = 1024
B = 8
F32 = mybir.dt.float32
AF = mybir.ActivationFunctionType
ALU = mybir.AluOpType

_CACHE: dict[int, object] = {}
TRACE = False
LAST_RESULT = None


def _qchunks(lv):
    out = []
    s = 0
    while s < lv:
        w = min(512, lv - s)
        out.append((s, w))
        s += w
    return out


def _build(lv):
    nj = lv // 128
    qch = _qchunks(lv)
    nqc = len(qch)

    nc = bacc.Bacc("TRN2", target_bir_lowering=False, debug=False)

    xT_d = nc.declare_dram_parameter("xT", [E, lv], F32, isOutput=False)
    wqT_d = nc.declare_dram_parameter("wqT", [E, E], F32, isOutput=False)
    wkT_d = nc.declare_dram_parameter("wkT", [E, E], F32, isOutput=False)
    wvT_d = nc.declare_dram_parameter("wvT", [E, E], F32, isOutput=False)
    woT_d = nc.declare_dram_parameter("woT", [E, E], F32, isOutput=False)
    grows_d = nc.declare_dram_parameter("grows", [lv, L_FULL], F32, isOutput=False)
    gqkT_d = nc.declare_dram_parameter("gqkT", [lv, lv], F32, isOutput=False)
    maskc_d = nc.declare_dram_parameter("maskc", [128, nj], F32, isOutput=False)
    maskr_d = nc.declare_dram_parameter("maskr", [1, lv], F32, isOutput=False)
    pad8_d = nc.declare_dram_parameter("pad8", [128, 1], F32, isOutput=False)
    out_d = nc.declare_dram_parameter("out", [lv, E], F32, isOutput=True)

    with tile.TileContext(nc) as tc:
        with (
            tc.tile_pool(name="per", bufs=1) as per,
            tc.tile_pool(name="gwork", bufs=2) as gwork,
            tc.tile_pool(name="ework", bufs=3) as ework,
            tc.tile_pool(name="owork", bufs=2) as owork,
            tc.tile_pool(name="ps1", bufs=3, space="PSUM") as ps1,
            tc.tile_pool(name="ps2", bufs=2, space="PSUM") as ps2,
        ):
            # ---- persistent SBUF ----
            xT = per.tile([128, 4, lv], F32)
            wq = per.tile([128, 4, E], F32)
            wk = per.tile([128, 4, E], F32)
            wv = per.tile([128, 4, E], F32)
            wo = per.tile([128, 4, E], F32)
            maskc = per.tile([128, nj], F32)
            maskr = per.tile([1, lv], F32)
            pad8 = per.tile([128, 1], F32)
            qT = per.tile([128, 4, lv], F32)
            kT = per.tile([128, 4, lv], F32)
            v = per.tile([128, nj, E], F32)
            expG = per.tile([128, nj, lv], F32)
            UT = per.tile([128, 4, lv], F32)
            GVT = per.tile([128, 4, lv], F32)
            cacc = per.tile([128, nj, 4, nqc], F32)
            dsum = per.tile([128, nj], F32)
            ones = per.tile([128, 128], F32)
            outsb = per.tile([128, nj, E], F32)

            nc.vector.memset(ones[:], 1.0)

            nc.sync.dma_start(xT[:], xT_d.ap().rearrange("(c p) l -> p c l", p=128))
            nc.sync.dma_start(wq[:], wqT_d.ap().rearrange("(c p) o -> p c o", p=128))
            nc.sync.dma_start(wk[:], wkT_d.ap().rearrange("(c p) o -> p c o", p=128))
            nc.sync.dma_start(wv[:], wvT_d.ap().rearrange("(c p) o -> p c o", p=128))
            nc.sync.dma_start(wo[:], woT_d.ap().rearrange("(c p) o -> p c o", p=128))
            nc.sync.dma_start(maskc[:], maskc_d.ap())
            nc.sync.dma_start(maskr[:], maskr_d.ap())
            nc.sync.dma_start(pad8[:], pad8_d.ap())

            # ---- projections: qT/kT (E_out on partitions), v (tokens on partitions) ----
            for w_sb, dst in ((wq, qT), (wk, kT)):
                for oc in range(4):
                    for qs, qw in qch:
                        pst = ps1.tile([128, 512], F32, tag="ps1")
                        for ic in range(4):
                            nc.tensor.matmul(
                                pst[:, :qw],
                                w_sb[:, ic, oc * 128 : (oc + 1) * 128],
                                xT[:, ic, qs : qs + qw],
                                start=(ic == 0),
                                stop=(ic == 3),
                            )
                        nc.vector.tensor_copy(dst[:, oc, qs : qs + qw], pst[:, :qw])
            for lc in range(nj):
                pst = ps1.tile([128, 512], F32, tag="ps1")
                for ic in range(4):
                    nc.tensor.matmul(
                        pst[:],
                        xT[:, ic, lc * 128 : (lc + 1) * 128],
                        wv[:, ic, :],
                        start=(ic == 0),
                        stop=(ic == 3),
                    )
                nc.vector.tensor_copy(v[:, lc, :], pst[:])

            # ---- G row denominators (full 1024 columns) -> colscale ----
            for jr in range(nj):
                gr = gwork.tile([128, L_FULL], F32, tag="grow")
                nc.sync.dma_start(gr[:], grows_d.ap()[jr * 128 : (jr + 1) * 128, :])
                scr = gwork.tile([128, L_FULL], F32, tag="gscr")
                nc.scalar.activation(
                    scr[:], gr[:], AF.Exp, accum_out=dsum[:, jr : jr + 1]
                )
            colinv = per.tile([128, nj], F32)
            nc.vector.reciprocal(colinv[:], dsum[:])
            # transpose colinv columns to a row vector via identity-rhs matmuls
            ident = per.tile([128, 128], F32)
            make_identity(nc, ident[:])
            colr = per.tile([1, lv], F32)
            for j in range(nj):
                pscr = ps1.tile([1, 512], F32, tag="ps1")
                nc.tensor.matmul(
                    pscr[:, :128], colinv[:, j : j + 1], ident[:], start=True, stop=True
                )
                nc.vector.tensor_copy(colr[:, j * 128 : (j + 1) * 128], pscr[:, :128])

            # ---- exp(G^T) on compacted tokens ----
            for j in range(nj):
                gq = gwork.tile([128, lv], F32, tag="gqk")
                nc.sync.dma_start(gq[:], gqkT_d.ap()[j * 128 : (j + 1) * 128, :])
                nc.scalar.activation(expG[:, j, :], gq[:], AF.Exp)

            # ---- GV^T = v^T-chunks @ expG ----
            for c in range(4):
                for qs, qw in qch:
                    psgv = ps1.tile([128, 512], F32, tag="ps1")
                    for j in range(nj):
                        nc.tensor.matmul(
                            psgv[:, :qw],
                            v[:, j, c * 128 : (c + 1) * 128],
                            expG[:, j, qs : qs + qw],
                            start=(j == 0),
                            stop=(j == nj - 1),
                        )
                    nc.vector.tensor_copy(GVT[:, c, qs : qs + qw], psgv[:, :qw])

            # ---- rg = mask_k-weighted column sums of expG; gsum ----
            rg = per.tile([1, lv], F32)
            for qs, qw in qch:
                psrg = ps1.tile([1, 512], F32, tag="ps1")
                for j in range(nj):
                    nc.tensor.matmul(
                        psrg[:, :qw],
                        maskc[:, j : j + 1],
                        expG[:, j, qs : qs + qw],
                        start=(j == 0),
                        stop=(j == nj - 1),
                    )
                nc.vector.tensor_copy(rg[:, qs : qs + qw], psrg[:, :qw])
            rgm = per.tile([1, lv], F32)
            nc.vector.tensor_mul(rgm[:], rg[:], maskr[:])
            nc.vector.tensor_mul(rgm[:], rgm[:], colr[:])
            gs = per.tile([1, 1], F32)
            nc.vector.reduce_sum(gs[:], rgm[:], axis=mybir.AxisListType.X)
            psb = ps1.tile([128, 1], F32, tag="ps1")
            nc.tensor.matmul(psb[:], ones[0:1, :], gs[:], start=True, stop=True)
            gsb = per.tile([128, 1], F32)
            nc.vector.tensor_copy(gsb[:], psb[:])
            ginv = per.tile([128, 1], F32)
            nc.vector.reciprocal(ginv[:], gsb[:])
            # b = MIX * colinv / gsum   (q on partitions, layout [128, nj])
            bq = per.tile([128, nj], F32)
            nc.vector.tensor_scalar(bq[:], colinv[:], ginv[:], MIX, ALU.mult, ALU.mult)

            # ---- attention: S^T = tanh(k q^T/8); E = exp; U^T += v^T E ----
            for hp in range(4):
                hA = 2 * hp
                hB = 2 * hp + 1
                for qi, (qs, qw) in enumerate(qch):
                    psu = ps1.tile([128, 512], F32, tag="ps1")
                    for j in range(nj):
                        psS = ps2.tile([128, 2, 512], F32, tag="ps2")
                        nc.tensor.matmul(
                            psS[:, 0, :qw],
                            kT[0:64, hp, j * 128 : (j + 1) * 128],
                            qT[0:64, hp, qs : qs + qw],
                            start=True,
                            stop=True,
                            tile_position=(0, 0),
                        )
                        nc.tensor.matmul(
                            psS[:, 1, :qw],
                            kT[64:128, hp, j * 128 : (j + 1) * 128],
                            qT[64:128, hp, qs : qs + qw],
                            start=True,
                            stop=True,
                            tile_position=(64, 0),
                        )
                        tnh = ework.tile([128, 2, 512], F32, tag="tnh")
                        nc.scalar.activation(
                            tnh[:, :, :qw], psS[:, :, :qw], AF.Tanh, scale=SCALE
                        )
                        eT = ework.tile([128, 2, 512], F32, tag="eT")
                        nc.scalar.activation(
                            eT[:, :, :qw],
                            tnh[:, :, :qw],
                            AF.Exp,
                            accum_out=cacc[:, j, hp, qi : qi + 1],
                        )
                        nc.tensor.matmul(
                            psu[0:64, :qw],
                            v[:, j, hA * 64 : hA * 64 + 64],
                            eT[:, 0, :qw],
                            start=(j == 0),
                            stop=(j == nj - 1),
                            tile_position=(0, 0),
                        )
                        nc.tensor.matmul(
                            psu[64:128, :qw],
                            v[:, j, hB * 64 : hB * 64 + 64],
                            eT[:, 1, :qw],
                            start=(j == 0),
                            stop=(j == nj - 1),
                            tile_position=(0, 64),
                        )
                    nc.vector.tensor_copy(UT[:, hp, qs : qs + qw], psu[:, :qw])

            # ---- T = sum_k mask_k * (sum_{h,q} E - 8*npad)  -> a = MIX / T ----
            csum = per.tile([128, nj], F32)
            nc.vector.reduce_sum(csum[:], cacc[:], axis=mybir.AxisListType.XY)
            csub = per.tile([128, nj], F32)
            nc.vector.tensor_scalar(csub[:], csum[:], pad8[:], None, ALU.subtract)
            cm = per.tile([128, nj], F32)
            nc.vector.tensor_mul(cm[:], csub[:], maskc[:])
            tcol = per.tile([128, 1], F32)
            nc.vector.reduce_sum(tcol[:], cm[:], axis=mybir.AxisListType.X)
            psT = ps1.tile([128, 1], F32, tag="ps1")
            nc.tensor.matmul(psT[:], ones[:], tcol[:], start=True, stop=True)
            tb = per.tile([128, 1], F32)
            nc.vector.tensor_copy(tb[:], psT[:])
            tinv = per.tile([128, 1], F32)
            nc.vector.reciprocal(tinv[:], tb[:])
            aq = per.tile([128, 1], F32)
            nc.vector.tensor_scalar_mul(aq[:], tinv[:], MIX)

            # ---- output: out = a * (UT.T @ woT) + b_l * (GVT.T @ woT) ----
            for lc in range(nj):
                psU = ps1.tile([128, 512], F32, tag="ps1")
                for c in range(4):
                    nc.tensor.matmul(
                        psU[:],
                        UT[:, c, lc * 128 : (lc + 1) * 128],
                        wo[:, c, :],
                        start=(c == 0),
                        stop=(c == 3),
                    )
                psG = ps1.tile([128, 512], F32, tag="ps1")
                for c in range(4):
                    nc.tensor.matmul(
                        psG[:],
                        GVT[:, c, lc * 128 : (lc + 1) * 128],
                        wo[:, c, :],
                        start=(c == 0),
                        stop=(c == 3),
                    )
                t2 = owork.tile([128, 512], F32, tag="t2")
                nc.vector.tensor_scalar_mul(t2[:], psG[:], bq[:, lc : lc + 1])
                nc.vector.scalar_tensor_tensor(
                    outsb[:, lc, :], psU[:], aq[:], t2[:], ALU.mult, ALU.add
                )
            nc.sync.dma_start(
                out_d.ap().rearrange("(nj p) f -> p nj f", p=128), outsb[:]
            )

    nc.compile()
    return nc


def kernel(x, mask, Wq, Wk, Wv, Wo, G):
    x = np.asarray(x, dtype=np.float32)
    maskb = np.asarray(mask).astype(bool)
    G = np.asarray(G, dtype=np.float32)
    b, l, e = x.shape

    idxs = [np.nonzero(maskb[i])[0] for i in range(b)]
    nmax = max((len(ix) for ix in idxs), default=0)
    lv = max(128, ((nmax + 127) // 128) * 128)

    if lv not in _CACHE:
        _CACHE[lv] = _build(lv)
    nc = _CACHE[lv]

    wqT = np.ascontiguousarray(np.asarray(Wq, np.float32).T)
    wkT = np.ascontiguousarray(np.asarray(Wk, np.float32).T)
    wvT = np.ascontiguousarray(np.asarray(Wv, np.float32).T)
    woT = np.ascontiguousarray(np.asarray(Wo, np.float32).T)

    in_maps = []
    for i in range(b):
        ix = idxs[i]
        n = len(ix)
        xT = np.zeros((E, lv), np.float32)
        xT[:, :n] = x[i][ix].T
        grows = np.zeros((lv, L_FULL), np.float32)
        grows[:n] = G[ix]
        gqkT = np.zeros((lv, lv), np.float32)
        gqkT[:n, :n] = G[np.ix_(ix, ix)].T
        mk = np.zeros(lv, np.float32)
        mk[:n] = 1.0
        maskc = np.ascontiguousarray(mk.reshape(lv // 128, 128).T)
        maskr = mk.reshape(1, lv)
        pad8 = np.full((128, 1), float(H * (lv - n)), np.float32)
        in_maps.append(
            {
                "xT": xT,
                "wqT": wqT,
                "wkT": wkT,
                "wvT": wvT,
                "woT": woT,
                "grows": grows,
                "gqkT": gqkT,
                "maskc": maskc,
                "maskr": maskr,
                "pad8": pad8,
            }
        )

    res = run_bass_kernel_spmd(nc, in_maps, list(range(B)), trace=TRACE)
    global LAST_RESULT
    LAST_RESULT = res

    out = np.zeros((b, l, e), np.float32)
    for i in range(b):
        ix = idxs[i]
        n = len(ix)
        if n:
            ov = res.results[i]["out"]
            out[i][ix] = ov.reshape(lv, E)[:n]
    return out
